# revision 1
# baseline (speedup 1.0000x reference)
"""AttentionBlock (GroupNorm + single-head LxL attention + residual) on 8 NeuronCores.

Sharding: data-parallel over batch B=8 -> one batch element per core.

Per-core strategy (C=512 channels, L=4096 positions):
  - GroupNorm stats on DVE+ACT; group reduction across the 16-channel blocks via
    two tiny matmuls against 0/1 group-map matrices; apply as per-partition affine.
  - All heavy matmuls run as fp8-e4m3 DoubleRow (K=256 per pass, ~2x bf16):
    channel dim packed as [Ki=128, j, o] with c = 256j + 128o + ki.
  - v is computed directly transposed (vT [L, C]) so attn@V contracts keys on
    the partition dim with no on-device transposes.
  - scores computed transposed: sT[m, l] = k^T q (keys on partitions); softmax
    over keys; exp emits p/16 so fp8's 448 max is never hit (cancels in the
    normalization); denominator via ones DoubleRow matmul; division deferred
    to the [C, L]-shaped attention output (128x less work than dividing p).
  - residual + biases fused into the PSUM->SBUF eviction (DVE scalar_tensor_tensor).
"""

import numpy as np
import ml_dtypes

import concourse.bass as bass
import concourse.bacc as bacc
import concourse.mybir as mybir
import concourse.tile as tile
from concourse.bass_utils import run_bass_kernel_spmd

F32 = mybir.dt.float32
BF16 = mybir.dt.bfloat16
FP8 = mybir.dt.float8e4
AF = mybir.ActivationFunctionType
ALU = mybir.AluOpType
AX = mybir.AxisListType
DR = mybir.MatmulPerfMode.DoubleRow

B = 8
C = 512
H = 64
W = 64
L = H * W          # 4096
G = 32             # groups
GSZ = C // G       # 16 channels per group
CT = C // 128      # 4 channel tiles
LC = L // 512      # 8 query chunks of 512
MT = L // 128      # 32 key tiles of 128
JM = MT // 2       # 16 DoubleRow key passes
NORM = 1.0 / (GSZ * L)   # 1/65536
EPS = 1e-5
ISQ = 1.0 / np.sqrt(np.float32(C))
LN16 = float(np.log(16.0))
DR_SPLIT = True  # walrus dual-fp8 wants the pair dim at ISA mem-pattern dim[2]


def _dr8(t):
    # ones lhsT for the denominator matmul: free dims (2, 1, 1) with the pair
    # dim outermost and a 16-element stride (LDW wants step_elem[2] %% 16 == 0)
    if not DR_SPLIT:
        return t[:, :, 0:1]
    return t[:, :, 0:1].rearrange("p o (a b) -> p o a b", a=1)


def _dr(ap):
    # [128, 2, M] -> [128, 2, 2, M//2]: pair dim ends up outermost of 3 free
    # dims = ISA dim[2] (s3_lw/s3d3_mm dual_fp8_restrictions). Element order
    # is unchanged, so semantics are identical.
    if not DR_SPLIT:
        return ap
    return ap.rearrange("p o (a b) -> p o a b", a=2)


def _build_nc():
    nc = bacc.Bacc("TRN2", target_bir_lowering=False, debug=False, num_devices=B)

    xb_d = nc.dram_tensor("xb", (C, L), BF16, kind="ExternalInput").ap()
    xf_d = nc.dram_tensor("xf", (C, L), F32, kind="ExternalInput").ap()
    # packed fp8 weights: [ki, j, o, cout] with cin = 256j + 128o + ki
    wq_d = nc.dram_tensor("wq8", (128, 2, 2, C), FP8, kind="ExternalInput").ap()
    wk_d = nc.dram_tensor("wk8", (128, 2, 2, C), FP8, kind="ExternalInput").ap()
    wv_d = nc.dram_tensor("wv8", (128, 2, 2, C), FP8, kind="ExternalInput").ap()
    wo_d = nc.dram_tensor("wo8", (128, 2, 2, C), FP8, kind="ExternalInput").ap()
    bq_d = nc.dram_tensor("bq", (128, CT), F32, kind="ExternalInput").ap()
    bk_d = nc.dram_tensor("bk", (128, CT), F32, kind="ExternalInput").ap()
    ob_d = nc.dram_tensor("ob", (128, CT), F32, kind="ExternalInput").ap()
    gam_d = nc.dram_tensor("gam", (128, CT), F32, kind="ExternalInput").ap()
    bet_d = nc.dram_tensor("bet", (128, CT), F32, kind="ExternalInput").ap()
    gmap_d = nc.dram_tensor("gmap", (128, 8), F32, kind="ExternalInput").ap()
    gmapT_d = nc.dram_tensor("gmapT", (8, 128), F32, kind="ExternalInput").ap()
    out_d = nc.dram_tensor("out", (C, L), F32, kind="ExternalOutput").ap()

    with tile.TileContext(nc) as tc:
        with (
            tc.tile_pool(name="wts", bufs=1) as wp,
            tc.tile_pool(name="small", bufs=1) as sp,
            tc.tile_pool(name="stats", bufs=4) as stp,
        ):
            # ---- constants / weights ----
            wq_t = wp.tile([128, 2, 2, C], FP8, tag="wq")
            wk_t = wp.tile([128, 2, 2, C], FP8, tag="wk")
            wv_t = wp.tile([128, 2, 2, C], FP8, tag="wv")
            wo_t = wp.tile([128, 2, 2, C], FP8, tag="wo")
            nc.sync.dma_start(wq_t[:], wq_d[:])
            nc.sync.dma_start(wk_t[:], wk_d[:])
            nc.sync.dma_start(wv_t[:], wv_d[:])
            nc.sync.dma_start(wo_t[:], wo_d[:])
            bq_t = sp.tile([128, CT], F32, tag="bq")
            bk_t = sp.tile([128, CT], F32, tag="bk")
            ob_t = sp.tile([128, CT], F32, tag="ob")
            gam_t = sp.tile([128, CT], F32, tag="gam")
            bet_t = sp.tile([128, CT], F32, tag="bet")
            gmap_t = sp.tile([128, 8], F32, tag="gmap")
            gmapT_t = sp.tile([8, 128], F32, tag="gmapT")
            nc.sync.dma_start(bq_t[:], bq_d[:])
            nc.sync.dma_start(bk_t[:], bk_d[:])
            nc.sync.dma_start(ob_t[:], ob_d[:])
            nc.sync.dma_start(gam_t[:], gam_d[:])
            nc.sync.dma_start(bet_t[:], bet_d[:])
            nc.sync.dma_start(gmap_t[:], gmap_d[:])
            nc.sync.dma_start(gmapT_t[:], gmapT_d[:])
            ones_8 = sp.tile([128, 2, 16], FP8, tag="ones_8")
            ones_r = sp.tile([1, 128], F32, tag="ones_r")
            eps_t = sp.tile([128, 1], F32, tag="eps")
            nsh_t = sp.tile([128, 1], F32, tag="nsh")
            nc.vector.memset(ones_8[:], 1.0)
            nc.vector.memset(ones_r[:], 1.0)
            nc.vector.memset(eps_t[:], EPS)
            nc.vector.memset(nsh_t[:], -LN16)

            with tc.tile_pool(name="qkv", bufs=1) as qkvp:
                # packed fp8: [ki, j, o, *] with channel c = 256j + 128o + ki
                q_t = qkvp.tile([128, 2, 2, L], FP8, tag="q")
                k_t = qkvp.tile([128, 2, 2, L], FP8, tag="k")
                vT_t = qkvp.tile([128, JM, 2, 512], FP8, tag="vT")

                # ---- phase 1: load x (bf16) + GroupNorm -> h8 (packed fp8) ----
                with tc.tile_pool(name="xh", bufs=1) as xhp:
                    x_t = xhp.tile([128, CT, L], BF16, tag="x")
                    h_t = xhp.tile([128, 2, 2, L], FP8, tag="h8")
                    for i in range(CT):
                        nc.sync.dma_start(x_t[:, i, :], xb_d[i * 128:(i + 1) * 128, :])
                    with (
                        tc.tile_pool(name="sq", bufs=2) as sqp,
                        tc.tile_pool(name="psg", bufs=2, space="PSUM") as psg,
                    ):
                        for i in range(CT):
                            st = stp.tile([128, 2], F32, tag="st")
                            sq = sqp.tile([128, L], BF16, tag="sq")
                            # st[:,0] = sum(x) on DVE, st[:,1] = sum(x^2) on ACT
                            nc.vector.reduce_sum(st[:, 0:1], x_t[:, i, :], axis=AX.X)
                            nc.scalar.activation(sq[:], x_t[:, i, :], AF.Square,
                                                 accum_out=st[:, 1:2])
                            gs_ps = psg.tile([8, 2], F32, tag="gs")
                            nc.tensor.matmul(gs_ps[:], gmap_t[:], st[:],
                                             start=True, stop=True)
                            gs_sb = stp.tile([8, 2], F32, tag="gssb")
                            nc.scalar.copy(gs_sb[:], gs_ps[:])
                            gb_ps = psg.tile([128, 2], F32, tag="gb")
                            nc.tensor.matmul(gb_ps[:], gmapT_t[:], gs_sb[:],
                                             start=True, stop=True)
                            nmean = stp.tile([128, 1], F32, tag="nmean")
                            ex2 = stp.tile([128, 1], F32, tag="ex2")
                            nc.vector.tensor_scalar_mul(nmean[:], gb_ps[:, 0:1], -NORM)
                            nc.vector.tensor_scalar_mul(ex2[:], gb_ps[:, 1:2], NORM)
                            msq = stp.tile([128, 1], F32, tag="msq")
                            var = stp.tile([128, 1], F32, tag="var")
                            nc.vector.tensor_mul(msq[:], nmean[:], nmean[:])
                            nc.vector.tensor_sub(var[:], ex2[:], msq[:])
                            std = stp.tile([128, 1], F32, tag="std")
                            nc.scalar.activation(std[:], var[:], AF.Sqrt, bias=eps_t[:])
                            rstd = stp.tile([128, 1], F32, tag="rstd")
                            nc.vector.reciprocal(rstd[:], std[:])
                            sc = stp.tile([128, 1], F32, tag="sc")
                            bc = stp.tile([128, 1], F32, tag="bc")
                            nc.vector.tensor_mul(sc[:], gam_t[:, i:i + 1], rstd[:])
                            nc.vector.scalar_tensor_tensor(
                                bc[:], nmean[:], sc[:], bet_t[:, i:i + 1],
                                ALU.mult, ALU.add)
                            if i % 2 == 0:
                                nc.scalar.activation(
                                    h_t[:, i // 2, i % 2, :], x_t[:, i, :],
                                    AF.Identity, bias=bc[:], scale=sc[:])
                            else:
                                nc.vector.tensor_scalar(
                                    h_t[:, i // 2, i % 2, :], x_t[:, i, :],
                                    sc[:], bc[:], ALU.mult, ALU.add)

                    # ---- phase 2: q, k, vT projections (fp8 DoubleRow) ----
                    with tc.tile_pool(name="psq", bufs=6, space="PSUM") as psq:
                        for ct in range(CT):
                            csl = slice(ct * 128, (ct + 1) * 128)
                            for lc in range(LC):
                                lsl = slice(lc * 512, (lc + 1) * 512)
                                ps = psq.tile([128, 512], F32, tag="ps")
                                for j in range(2):
                                    nc.tensor.matmul(
                                        ps[:], _dr(wq_t[:, j, :, csl]), _dr(h_t[:, j, :, lsl]),
                                        start=(j == 0), stop=(j == 1), perf_mode=DR)
                                nc.vector.tensor_scalar_add(
                                    q_t[:, ct // 2, ct % 2, lsl], ps[:],
                                    bq_t[:, ct:ct + 1])
                                ps2 = psq.tile([128, 512], F32, tag="ps")
                                for j in range(2):
                                    nc.tensor.matmul(
                                        ps2[:], _dr(wk_t[:, j, :, csl]), _dr(h_t[:, j, :, lsl]),
                                        start=(j == 0), stop=(j == 1), perf_mode=DR)
                                nc.scalar.activation(
                                    k_t[:, ct // 2, ct % 2, lsl], ps2[:],
                                    AF.Identity, bias=bk_t[:, ct:ct + 1])
                        for mt in range(MT):
                            msl = slice(mt * 128, (mt + 1) * 128)
                            ps = psq.tile([128, 512], F32, tag="ps")
                            for j in range(2):
                                nc.tensor.matmul(
                                    ps[:], _dr(h_t[:, j, :, msl]), _dr(wv_t[:, j, :, :]),
                                    start=(j == 0), stop=(j == 1), perf_mode=DR)
                            if mt % 2 == 0:
                                nc.scalar.copy(vT_t[:, mt // 2, mt % 2, :], ps[:])
                            else:
                                nc.vector.tensor_copy(
                                    vT_t[:, mt // 2, mt % 2, :], ps[:])
                # xh pool closed: x/h SBUF reclaimed before attention buffers open

                # ---- phase 3+4: attention + out-projection, per query chunk ----
                with (
                    tc.tile_pool(name="at", bufs=1) as atp,
                    tc.tile_pool(name="pp", bufs=1) as ppool,
                    tc.tile_pool(name="den", bufs=1) as dpool,
                    tc.tile_pool(name="psa", bufs=1, space="PSUM") as psa,
                    tc.tile_pool(name="xo", bufs=4) as xop,
                ):
                    at_t = atp.tile([128, 2, 2, L], FP8, tag="at")
                    for lc in range(LC):
                        lsl = slice(lc * 512, (lc + 1) * 512)
                        ops = [psa.tile([128, 512], F32, tag=f"o{ct}", bufs=1,
                                        name=f"ops{ct}_{lc}")
                               for ct in range(CT)]
                        den_ps = psa.tile([1, 512], F32, tag="bc", bufs=1)
                        p8 = ppool.tile([128, JM, 2, 512], FP8, tag="p",
                                        bufs=3, name=f"p8_{lc}")
                        for mt in range(MT):
                            msl = slice(mt * 128, (mt + 1) * 128)
                            sps = psa.tile([128, 512], F32, tag="sps", bufs=3)
                            for j in range(2):
                                nc.tensor.matmul(
                                    sps[:], _dr(k_t[:, j, :, msl]), _dr(q_t[:, j, :, lsl]),
                                    start=(j == 0), stop=(j == 1), perf_mode=DR)
                            # p = exp(s/sqrt(C))/16 : stays well inside fp8 range
                            nc.scalar.activation(p8[:, mt // 2, mt % 2, :], sps[:],
                                                 AF.Exp, bias=nsh_t[:], scale=ISQ)
                            if mt % 2 == 1:
                                jm = mt // 2
                                nc.tensor.matmul(
                                    den_ps[:], _dr8(ones_8), _dr(p8[:, jm, :, :]),
                                    start=(jm == 0), stop=(jm == JM - 1),
                                    perf_mode=DR)
                                for ct in range(CT):
                                    nc.tensor.matmul(
                                        ops[ct][:],
                                        _dr(vT_t[:, jm, :, ct * 128:(ct + 1) * 128]),
                                        _dr(p8[:, jm, :, :]),
                                        start=(jm == 0), stop=(jm == JM - 1),
                                        perf_mode=DR)
                        rec = dpool.tile([1, 512], F32, tag="rec")
                        nc.vector.reciprocal(rec[:], den_ps[:])
                        bc_ps = psa.tile([128, 512], F32, tag="bc", bufs=1)
                        nc.tensor.matmul(bc_ps[:], ones_r[:], rec[:],
                                         start=True, stop=True)
                        bc_sb = dpool.tile([128, 512], F32, tag="bcsb")
                        nc.scalar.copy(bc_sb[:], bc_ps[:])
                        for ct in range(CT):
                            nc.vector.tensor_mul(
                                at_t[:, ct // 2, ct % 2, lsl], ops[ct][:], bc_sb[:])

                        # out projection + bias(wo@bv+bo) + residual for this
                        # chunk; reuses the freed "bc" PSUM slot so it overlaps
                        # the next chunk's attention matmuls
                        for ct in range(CT):
                            csl = slice(ct * 128, (ct + 1) * 128)
                            ps = psa.tile([128, 512], F32, tag="bc", bufs=1,
                                          name=f"ops_o_{ct}_{lc}")
                            for j in range(2):
                                nc.tensor.matmul(
                                    ps[:], _dr(wo_t[:, j, :, csl]), _dr(at_t[:, j, :, lsl]),
                                    start=(j == 0), stop=(j == 1), perf_mode=DR)
                            xr = xop.tile([128, 512], F32, tag="xr")
                            nc.sync.dma_start(xr[:], xf_d[csl, lsl])
                            osb = xop.tile([128, 512], F32, tag="osb")
                            nc.vector.scalar_tensor_tensor(
                                osb[:], ps[:], ob_t[:, ct:ct + 1], xr[:],
                                ALU.add, ALU.add)
                            nc.sync.dma_start(out_d[csl, lsl], osb[:])

    nc.compile()
    return nc


_NC_CACHE = {}
PROFILE = False
LAST_RESULT = {}


def _get_nc():
    if "nc" not in _NC_CACHE:
        _NC_CACHE["nc"] = _build_nc()
    return _NC_CACHE["nc"]


def _pack_w(w):
    # w: (Cout, Cin) fp32 -> packed lhsT [ki, j, o, Cout] fp8, cin = 256j+128o+ki
    f8 = mybir.dt.np(FP8)
    wT = np.asarray(w, np.float32).T.reshape(2, 2, 128, C)  # [j, o, ki, cout]
    return np.ascontiguousarray(wT.transpose(2, 0, 1, 3)).astype(f8)


def kernel(x, gn_gamma, gn_beta, wq, bq, wk, bk, wv, bv, wo, bo):
    x = np.asarray(x, np.float32)
    bf = ml_dtypes.bfloat16

    def fold(v):  # (512,) -> (128, 4) where [:, ct] = v[128*ct : 128*(ct+1)]
        return np.ascontiguousarray(np.asarray(v, np.float32).reshape(CT, 128).T)

    ob = fold(np.asarray(wo, np.float32) @ np.asarray(bv, np.float32)
              + np.asarray(bo, np.float32))
    gmap = np.zeros((128, 8), np.float32)
    gmap[np.arange(128), np.arange(128) // GSZ] = 1.0
    shared = {
        "wq8": _pack_w(wq), "wk8": _pack_w(wk), "wv8": _pack_w(wv),
        "wo8": _pack_w(wo),
        "bq": fold(bq), "bk": fold(bk), "ob": ob,
        "gam": fold(gn_gamma), "bet": fold(gn_beta),
        "gmap": gmap, "gmapT": np.ascontiguousarray(gmap.T),
    }
    in_maps = []
    for b in range(B):
        xb = np.ascontiguousarray(x[b].reshape(C, L))
        in_maps.append({"xb": xb.astype(bf), "xf": xb, **shared})

    nc = _get_nc()
    res = run_bass_kernel_spmd(nc, in_maps, list(range(B)), trace=PROFILE)
    LAST_RESULT["res"] = res
    out = np.stack([res.results[b]["out"].reshape(C, H, W) for b in range(B)])
    return out.astype(np.float32)



# revision 19
# speedup vs baseline: 1.2290x; 1.2290x over previous
"""AttentionBlock (GroupNorm + single-head LxL attention + residual) on 8 NeuronCores.

Sharding: data-parallel over batch B=8 -> one batch element per core.

Per-core strategy (C=512 channels, L=4096 positions):
  - GroupNorm stats on DVE (sums) + ACT (squares); group reduction via two
    tiny matmuls against 0/1 group-map matrices; per-partition affine apply.
  - All heavy matmuls run as fp8-e4m3 DoubleRow (K=256 per pass, ~2x bf16):
    channel dim packed as [Ki=128, j, o] with c = 256j + 128o + ki.
  - v is computed directly transposed (vT [L, C]) so attn@V contracts keys on
    the partition dim with no on-device transposes.
  - scores computed transposed: sT[m, l] = k^T q (keys on partitions); softmax
    over keys; exp emits p/16 so fp8's 448 max is never hit (cancels in the
    normalization). The Activation engine is the attention bottleneck, so exp
    runs on [128,1024] double-bank PSUM tiles (16/chunk) and ACT does nothing
    else during attention.
  - PSUM is four [128,1024] double-bank slot groups: "sps" (2 bufs, score
    ping-pong), "oAB"/"oCD" (PV accumulators ct0/1 and ct2/3 as half-tile
    regions). QKV projections rotate over all four groups so the k (ACT) and
    q (DVE) PSUM evictions drain concurrently.
  - denominator via ones DoubleRow matmuls into a score-rotation slot after
    each chunk's loop; division deferred to the [C, L] attention output.
  - chunk tails (1/den broadcast, at-quantize, out-projection, residual) are
    software-pipelined into the next chunk so PE never idles (PE has a
    p-state ramp: any idle gap halves matmul speed for ~3us).
  - residual added from the bf16 x kept resident in SBUF (no f32 re-load).
"""

import os
import numpy as np
import ml_dtypes

import concourse.bass as bass
import concourse.bacc as bacc
import concourse.mybir as mybir
import concourse.tile as tile
from concourse.bass_utils import run_bass_kernel_spmd

F32 = mybir.dt.float32
BF16 = mybir.dt.bfloat16
FP8 = mybir.dt.float8e4
AF = mybir.ActivationFunctionType
ALU = mybir.AluOpType
AX = mybir.AxisListType
DR = mybir.MatmulPerfMode.DoubleRow

B = 8
C = 512
H = 64
W = 64
L = H * W          # 4096
G = 32             # groups
GSZ = C // G       # 16 channels per group
CT = C // 128      # 4 channel tiles
LC = L // 512      # 8 query chunks of 512
MT = L // 128      # 32 key tiles of 128
JM = MT // 2       # 16 DoubleRow key passes
NORM = 1.0 / (GSZ * L)   # 1/65536
EPS = 1e-5
ISQ = 1.0 / np.sqrt(np.float32(C))
LN16 = float(np.log(16.0))
DR_SPLIT = True  # walrus dual-fp8 wants the pair dim at ISA mem-pattern dim[2]

PV_LAG = int(os.environ.get("K_PV_LAG", "7"))
B_PVB = int(os.environ.get("K_B_PVB", "0"))
B_DEN = int(os.environ.get("K_B_DEN", "1"))
B_FLUSH = int(os.environ.get("K_B_FLUSH", "2"))


def _dr8(t):
    # ones lhsT for the denominator matmul: free dims (2, 1, 1) with the pair
    # dim outermost and a 16-element stride (LDW wants step_elem[2] %% 16 == 0)
    if not DR_SPLIT:
        return t[:, :, 0:1]
    return t[:, :, 0:1].rearrange("p o (a b) -> p o a b", a=1)


def _dr(ap):
    # [128, 2, M] -> [128, 2, 2, M//2]: pair dim ends up outermost of 3 free
    # dims = ISA dim[2] (s3_lw/s3d3_mm dual_fp8_restrictions). Element order
    # is unchanged, so semantics are identical.
    if not DR_SPLIT:
        return ap
    return ap.rearrange("p o (a b) -> p o a b", a=2)


def _build_nc():
    nc = bacc.Bacc("TRN2", target_bir_lowering=False, debug=False, num_devices=B)

    xb_d = nc.dram_tensor("xb", (C, L), BF16, kind="ExternalInput").ap()
    # packed fp8 weights: [ki, j, o, cout] with cin = 256j + 128o + ki
    wq_d = nc.dram_tensor("wq8", (128, 2, 2, C), FP8, kind="ExternalInput").ap()
    wk_d = nc.dram_tensor("wk8", (128, 2, 2, C), FP8, kind="ExternalInput").ap()
    wv_d = nc.dram_tensor("wv8", (128, 2, 2, C), FP8, kind="ExternalInput").ap()
    wo_d = nc.dram_tensor("wo8", (128, 2, 2, C), FP8, kind="ExternalInput").ap()
    bq_d = nc.dram_tensor("bq", (128, CT), F32, kind="ExternalInput").ap()
    bk_d = nc.dram_tensor("bk", (128, CT), F32, kind="ExternalInput").ap()
    ob_d = nc.dram_tensor("ob", (128, CT), F32, kind="ExternalInput").ap()
    gam_d = nc.dram_tensor("gam", (128, CT), F32, kind="ExternalInput").ap()
    bet_d = nc.dram_tensor("bet", (128, CT), F32, kind="ExternalInput").ap()
    gmap_d = nc.dram_tensor("gmap", (128, 8), F32, kind="ExternalInput").ap()
    gmapT_d = nc.dram_tensor("gmapT", (8, 128), F32, kind="ExternalInput").ap()
    out_d = nc.dram_tensor("out", (C, L), F32, kind="ExternalOutput").ap()

    with tile.TileContext(nc) as tc:
        with (
            tc.tile_pool(name="wts", bufs=1) as wp,
            tc.tile_pool(name="small", bufs=1) as sp,
            tc.tile_pool(name="stats", bufs=4) as stp,
            tc.tile_pool(name="xres", bufs=1) as xpp,
            tc.tile_pool(name="qkv", bufs=1) as qkvp,
        ):
            # ---- x first (GN is the critical path), then weights ----
            x_t = xpp.tile([128, CT, L], BF16, tag="x")
            for i in range(CT):
                for hf in range(2):
                    nc.sync.dma_start(
                        x_t[:, i, hf * 2048:(hf + 1) * 2048],
                        xb_d[i * 128:(i + 1) * 128,
                             hf * 2048:(hf + 1) * 2048])
            wq_t = wp.tile([128, 2, 2, C], FP8, tag="wq")
            wk_t = wp.tile([128, 2, 2, C], FP8, tag="wk")
            wv_t = wp.tile([128, 2, 2, C], FP8, tag="wv")
            wo_t = wp.tile([128, 2, 2, C], FP8, tag="wo")
            nc.sync.dma_start(wk_t[:], wk_d[:])
            nc.sync.dma_start(wq_t[:], wq_d[:])
            nc.sync.dma_start(wv_t[:], wv_d[:])
            nc.sync.dma_start(wo_t[:], wo_d[:])
            bq_t = sp.tile([128, CT], F32, tag="bq")
            bk_t = sp.tile([128, CT], F32, tag="bk")
            ob_t = sp.tile([128, CT], F32, tag="ob")
            gam_t = sp.tile([128, CT], F32, tag="gam")
            bet_t = sp.tile([128, CT], F32, tag="bet")
            gmap_t = sp.tile([128, 8], F32, tag="gmap")
            gmapT_t = sp.tile([8, 128], F32, tag="gmapT")
            nc.sync.dma_start(gam_t[:], gam_d[:])
            nc.sync.dma_start(bet_t[:], bet_d[:])
            nc.sync.dma_start(gmap_t[:], gmap_d[:])
            nc.sync.dma_start(gmapT_t[:], gmapT_d[:])
            nc.sync.dma_start(bq_t[:], bq_d[:])
            nc.sync.dma_start(bk_t[:], bk_d[:])
            nc.sync.dma_start(ob_t[:], ob_d[:])
            ones_8 = sp.tile([128, 2, 16], FP8, tag="ones_8")
            ones_r = sp.tile([1, 128], BF16, tag="ones_r")
            eps_t = sp.tile([128, 1], F32, tag="eps")
            nsh_t = sp.tile([128, 1], F32, tag="nsh")
            nc.vector.memset(ones_8[:], 1.0)
            nc.vector.memset(ones_r[:], 1.0)
            nc.vector.memset(eps_t[:], EPS)
            nc.vector.memset(nsh_t[:], -LN16)

            # packed fp8: [ki, j, o, *] with channel c = 256j + 128o + ki
            q_t = qkvp.tile([128, 2, 2, L], FP8, tag="q")
            k_t = qkvp.tile([128, 2, 2, L], FP8, tag="k")
            vT_t = qkvp.tile([128, JM, 2, 512], FP8, tag="vT")
            h_t = qkvp.tile([128, 2, 2, L], FP8, tag="h8")

            # One PSUM pool, four [128,1024] double-bank slot groups.
            with (
                tc.tile_pool(name="psa", bufs=1, space="PSUM") as psa,
                tc.tile_pool(name="sq", bufs=2) as sqp,
            ):
                def big_tile(tag, name):
                    return psa.tile([128, 1024], F32, tag=tag, bufs=2
                                    if tag == "sps" else 1, name=name)

                # ---- phase 1: GroupNorm -> h8 (packed fp8) ----
                affs = []
                for i in range(CT):
                    st = stp.tile([128, 2], F32, tag="st")
                    sq = sqp.tile([128, L], BF16, tag="sq")
                    # st[:,0] = sum(x) on DVE, st[:,1] = sum(x^2) on ACT
                    nc.vector.reduce_sum(st[:, 0:1], x_t[:, i, :], axis=AX.X)
                    nc.scalar.activation(sq[:], x_t[:, i, :], AF.Square,
                                         accum_out=st[:, 1:2])
                    gs_ps = big_tile("oAB", f"gs_{i}")
                    nc.tensor.matmul(gs_ps[0:8, 0:2], gmap_t[:], st[:],
                                     start=True, stop=True)
                    gs_sb = stp.tile([8, 2], F32, tag="gssb")
                    nc.scalar.copy(gs_sb[:], gs_ps[0:8, 0:2])
                    gb_ps = big_tile("oCD", f"gb_{i}")
                    nc.tensor.matmul(gb_ps[:, 0:2], gmapT_t[:], gs_sb[:],
                                     start=True, stop=True)
                    nmean = stp.tile([128, 1], F32, tag="nmean")
                    ex2 = stp.tile([128, 1], F32, tag="ex2")
                    nc.vector.tensor_scalar_mul(nmean[:], gb_ps[:, 0:1], -NORM)
                    nc.vector.tensor_scalar_mul(ex2[:], gb_ps[:, 1:2], NORM)
                    msq = stp.tile([128, 1], F32, tag="msq")
                    var = stp.tile([128, 1], F32, tag="var")
                    nc.vector.tensor_mul(msq[:], nmean[:], nmean[:])
                    nc.vector.tensor_sub(var[:], ex2[:], msq[:])
                    std = stp.tile([128, 1], F32, tag="std")
                    nc.scalar.activation(std[:], var[:], AF.Sqrt, bias=eps_t[:])
                    rstd = stp.tile([128, 1], F32, tag="rstd")
                    nc.vector.reciprocal(rstd[:], std[:])
                    sc = stp.tile([128, 1], F32, tag="sc")
                    bc = stp.tile([128, 1], F32, tag="bc")
                    nc.vector.tensor_mul(sc[:], gam_t[:, i:i + 1], rstd[:])
                    nc.vector.scalar_tensor_tensor(
                        bc[:], nmean[:], sc[:], bet_t[:, i:i + 1],
                        ALU.mult, ALU.add)
                    affs.append((sc, bc))
                # affines after all stats so the last tile's chain isn't
                # queued behind earlier affines; alternate ACT/DVE.
                for i in range(CT):
                    sc, bc = affs[i]
                    if i % 2 == 0:
                        nc.scalar.activation(
                            h_t[:, i // 2, i % 2, :], x_t[:, i, :],
                            AF.Identity, bias=bc[:], scale=sc[:])
                    else:
                        nc.vector.tensor_scalar(
                            h_t[:, i // 2, i % 2, :], x_t[:, i, :],
                            sc[:], bc[:], ALU.mult, ALU.add)

                # ---- phase 2: k, q, vT projections (fp8 DoubleRow) ----
                # [128,1024] PSUM tiles rotating over all four slot groups;
                # ACT evictions (k, half of vT) and DVE evictions (q, other
                # half of vT) drain concurrently.
                rot = ["sps", "oAB", "sps", "oCD"]
                rot_i = [0]

                def proj_tile(name):
                    t = big_tile(rot[rot_i[0] % 4], name)
                    rot_i[0] += 1
                    return t

                def kq_tile(w8, b_t, dst, ct, lp, evict):
                    csl = slice(ct * 128, (ct + 1) * 128)
                    lsl2 = slice(lp * 1024, (lp + 1) * 1024)
                    ps = proj_tile(f"kq_{ct}_{lp}")
                    for hh in range(2):
                        lsl = slice((2 * lp + hh) * 512, (2 * lp + hh + 1) * 512)
                        for j in range(2):
                            nc.tensor.matmul(
                                ps[:, hh * 512:(hh + 1) * 512],
                                _dr(w8[:, j, :, csl]), _dr(h_t[:, j, :, lsl]),
                                start=(j == 0), stop=(j == 1), perf_mode=DR)
                    if evict == "act":
                        nc.scalar.activation(dst[:, ct // 2, ct % 2, lsl2],
                                             ps[:], AF.Identity,
                                             bias=b_t[:, ct:ct + 1])
                    else:
                        nc.vector.tensor_scalar_add(
                            dst[:, ct // 2, ct % 2, lsl2], ps[:],
                            b_t[:, ct:ct + 1])

                def v_tile(jm, evict):
                    ps = proj_tile(f"vps_{jm}")
                    for o in range(2):
                        msl = slice((2 * jm + o) * 128, (2 * jm + o + 1) * 128)
                        for j in range(2):
                            nc.tensor.matmul(
                                ps[:, o * 512:(o + 1) * 512],
                                _dr(h_t[:, j, :, msl]), _dr(wv_t[:, j, :, :]),
                                start=(j == 0), stop=(j == 1), perf_mode=DR)
                    if evict == "act":
                        nc.scalar.copy(vT_t[:, jm, :, :], ps[:])
                    else:
                        nc.vector.tensor_copy(vT_t[:, jm, :, :], ps[:])

                # interleave: ACT-evicted k with DVE-evicted q, then vT split.
                for ct in range(CT):
                    for lp in range(LC // 2):
                        kq_tile(wk_t, bk_t, k_t, ct, lp, "act")
                        kq_tile(wq_t, bq_t, q_t, (ct + 1) % CT, lp, "dve")
                for jm in range(JM):
                    v_tile(jm, "act" if jm % 2 else "dve")

                # ---- phase 3+4: attention + out-projection, per query chunk ----
                with (
                    tc.tile_pool(name="at", bufs=2) as atp,
                    tc.tile_pool(name="pp", bufs=2) as ppool,
                    tc.tile_pool(name="den", bufs=2) as dpool,
                    tc.tile_pool(name="xo", bufs=4) as xop,
                ):
                    pending = {}

                    def flush_tail(p, last=False):
                        lc = p["lc"]
                        at_p = p["at"]
                        lsl = slice(lc * 512, (lc + 1) * 512)
                        dr_t = p["dr"]
                        nc.tensor.matmul(dr_t[:, 512:1024], ones_r[:],
                                         p["rec"][:], start=True, stop=True)
                        bc_sb = dpool.tile([128, 512], F32, tag="bcsb")
                        nc.vector.tensor_copy(bc_sb[:], dr_t[:, 512:1024])
                        opsAB, opsCD = p["opsAB"], p["opsCD"]

                        def at_mul(ct):
                            src = opsAB if ct < 2 else opsCD
                            rg = slice((ct % 2) * 512, (ct % 2) * 512 + 512)
                            nc.vector.tensor_mul(
                                at_p[:, ct // 2, ct % 2, :], src[:, rg],
                                bc_sb[:])

                        poAB = big_tile("oAB", f"po_{lc}_ab")
                        poCD = big_tile("oCD", f"po_{lc}_cd")

                        def po_pass(j):
                            for ct in range(CT):
                                csl = slice(ct * 128, (ct + 1) * 128)
                                po = poAB if ct < 2 else poCD
                                rg = slice((ct % 2) * 512, (ct % 2) * 512 + 512)
                                nc.tensor.matmul(
                                    po[:, rg], _dr(wo_t[:, j, :, csl]),
                                    _dr(at_p[:, j, :, :]),
                                    start=(j == 0), stop=(j == 1), perf_mode=DR)

                        # j-interleaved: out-proj j0 only needs at ct0/ct1
                        at_mul(0)
                        at_mul(1)
                        po_pass(0)
                        at_mul(2)
                        at_mul(3)
                        po_pass(1)
                        for ct in range(CT):
                            csl = slice(ct * 128, (ct + 1) * 128)
                            po = poAB if ct < 2 else poCD
                            rg = slice((ct % 2) * 512, (ct % 2) * 512 + 512)
                            osb = xop.tile([128, 512], F32, tag="osb")
                            if last and ct % 2 == 1:
                                nc.scalar.activation(
                                    osb[:], po[:, rg], AF.Identity,
                                    bias=ob_t[:, ct:ct + 1])
                                osb2 = xop.tile([128, 512], F32, tag="osb")
                                nc.vector.tensor_add(osb2[:], osb[:],
                                                     x_t[:, ct, lsl])
                                osb = osb2
                            else:
                                nc.vector.scalar_tensor_tensor(
                                    osb[:], po[:, rg], ob_t[:, ct:ct + 1],
                                    x_t[:, ct, lsl], ALU.add, ALU.add)
                            nc.sync.dma_start(out_d[csl, lsl], osb[:])

                    def pv_group(p, pv, last):
                        for ct in range(CT):
                            dst = p["opsAB"] if ct < 2 else p["opsCD"]
                            rg = slice((ct % 2) * 512, (ct % 2) * 512 + 512)
                            nc.tensor.matmul(
                                dst[:, rg],
                                _dr(vT_t[:, pv, :, ct * 128:(ct + 1) * 128]),
                                _dr(p["p8"][:, pv, :, :]),
                                start=(pv == 0), stop=last, perf_mode=DR)

                    def pv_backlog(p):
                        for pv in range(JM - PV_LAG, JM):
                            pv_group(p, pv, pv == JM - 1)

                    def emit_den(p):
                        # denominator: 16 ones-matmuls over the previous
                        # chunk's p8 into a score-rotation slot, then 1/den.
                        dr_t = psa.tile([128, 1024], F32, tag="sps", bufs=2,
                                        name=f"denrec_{p['lc']}")
                        for jm in range(JM):
                            nc.tensor.matmul(
                                dr_t[0:1, 0:512], _dr8(ones_8),
                                _dr(p["p8"][:, jm, :, :]),
                                start=(jm == 0), stop=(jm == JM - 1),
                                perf_mode=DR)
                        rec = dpool.tile([1, 512], BF16, tag="rec")
                        with nc.allow_low_precision(reason="1/den bcast bf16"):
                            nc.vector.reciprocal(rec[:], dr_t[0:1, 0:512])
                        p["dr"] = dr_t
                        p["rec"] = rec

                    cur = {}
                    for lc in range(LC):
                        lsl = slice(lc * 512, (lc + 1) * 512)
                        pending, cur = cur, {
                            "lc": lc,
                            "at": atp.tile([128, 2, 2, 512], FP8, tag="at",
                                           name=f"at_{lc}"),
                            "p8": ppool.tile([128, JM, 2, 512], FP8, tag="p",
                                             name=f"p8_{lc}"),
                        }
                        for jm in range(JM):
                            sps = psa.tile([128, 1024], F32, tag="sps", bufs=2,
                                           name=f"sps_{lc}_{jm}")
                            for o in range(2):
                                msl = slice((2 * jm + o) * 128,
                                            (2 * jm + o + 1) * 128)
                                for j in range(2):
                                    nc.tensor.matmul(
                                        sps[:, o * 512:(o + 1) * 512],
                                        _dr(k_t[:, j, :, msl]),
                                        _dr(q_t[:, j, :, lsl]),
                                        start=(j == 0), stop=(j == 1),
                                        perf_mode=DR)
                            # p = exp(s/sqrt(C))/16: inside fp8 range; both
                            # key sub-tiles in one [128,1024] ACT op.
                            nc.scalar.activation(cur["p8"][:, jm, :, :], sps[:],
                                                 AF.Exp, bias=nsh_t[:],
                                                 scale=ISQ)
                            if pending:
                                # previous chunk's deferred work, spread over
                                # this chunk's first blocks so the score
                                # pipeline never queues behind it.
                                if jm == B_PVB:
                                    pv_backlog(pending)
                                if jm == B_DEN:
                                    emit_den(pending)
                                if jm == B_FLUSH:
                                    flush_tail(pending)
                                    pending = {}
                            if jm == PV_LAG:
                                cur["opsAB"] = big_tile("oAB", f"ops_{lc}_ab")
                                cur["opsCD"] = big_tile("oCD", f"ops_{lc}_cd")
                            if jm >= PV_LAG:
                                pv_group(cur, jm - PV_LAG, False)
                    pv_backlog(cur)
                    emit_den(cur)
                    flush_tail(cur, last=True)

    nc.compile()
    return nc


_NC_CACHE = {}
PROFILE = False
LAST_RESULT = {}


def _get_nc():
    if "nc" not in _NC_CACHE:
        _NC_CACHE["nc"] = _build_nc()
    return _NC_CACHE["nc"]


def _pack_w(w):
    # w: (Cout, Cin) fp32 -> packed lhsT [ki, j, o, Cout] fp8, cin = 256j+128o+ki
    f8 = mybir.dt.np(FP8)
    wT = np.asarray(w, np.float32).T.reshape(2, 2, 128, C)  # [j, o, ki, cout]
    return np.ascontiguousarray(wT.transpose(2, 0, 1, 3)).astype(f8)


def kernel(x, gn_gamma, gn_beta, wq, bq, wk, bk, wv, bv, wo, bo):
    x = np.asarray(x, np.float32)
    bf = ml_dtypes.bfloat16

    def fold(v):  # (512,) -> (128, 4) where [:, ct] = v[128*ct : 128*(ct+1)]
        return np.ascontiguousarray(np.asarray(v, np.float32).reshape(CT, 128).T)

    ob = fold(np.asarray(wo, np.float32) @ np.asarray(bv, np.float32)
              + np.asarray(bo, np.float32))
    gmap = np.zeros((128, 8), np.float32)
    gmap[np.arange(128), np.arange(128) // GSZ] = 1.0
    shared = {
        "wq8": _pack_w(wq), "wk8": _pack_w(wk), "wv8": _pack_w(wv),
        "wo8": _pack_w(wo),
        "bq": fold(bq), "bk": fold(bk), "ob": ob,
        "gam": fold(gn_gamma), "bet": fold(gn_beta),
        "gmap": gmap, "gmapT": np.ascontiguousarray(gmap.T),
    }
    in_maps = []
    for b in range(B):
        xb = np.ascontiguousarray(x[b].reshape(C, L))
        in_maps.append({"xb": xb.astype(bf), **shared})

    nc = _get_nc()
    res = run_bass_kernel_spmd(nc, in_maps, list(range(B)), trace=PROFILE)
    LAST_RESULT["res"] = res
    out = np.stack([res.results[b]["out"].reshape(C, H, W) for b in range(B)])
    return out.astype(np.float32)
